# revision 2
# baseline (speedup 1.0000x reference)
"""Trainium2 Bass kernel for Disk descriptor mutual-NN matching (retrieval_knn).

Strategy (8 NeuronCores, shard descriptors1 columns M across cores):
  - Each core c holds full d0 [256, 8192] and its d1 shard [256, 1024].
  - Forward:  S_c = d0.T @ d1_c   -> per-row (N) top-8 values + indices over
    the core's 1024 local columns via the DVE top-8 instruction (InstMax /
    InstMaxIndex).  Host merges the 8 per-core top-2s into the global top-2.
  - Backward: S_c^T = d1_c.T @ d0 -> per-local-column top-8 over all 8192
    rows (full column reduction, complete on one core; no merge needed).
  - Host applies the exact Disk/SuperPoint ratio-test + mutual-NN logic in
    float32, replicating the reference's arithmetic (sqrt transform,
    division-based ratio test, tie semantics).

Matmuls run in native fp32 on the PE (exact-quality f32: measured max error
4.5e-8 vs f64 on-device, better than numpy f32).
"""

import sys

if "/opt/trn_rl_repo" not in sys.path:
    sys.path.insert(0, "/opt/trn_rl_repo")

import numpy as np

N_KPTS = 8192
M_KPTS = 8192
F_DIM = 256
N_CORES = 8
M_SHARD = M_KPTS // N_CORES  # 1024

SQRT_2 = np.float32(1.414213)
CLIP_LO = np.float32(1e-6)
ONE = np.float32(1.0)


# --------------------------------------------------------------------------
# Device kernel builder
# --------------------------------------------------------------------------

def build_kernel(n_rows=N_KPTS, m_shard=M_SHARD, f_dim=F_DIM, repeat=1):
    """Build the per-core SPMD Bass program.

    Inputs (per core):
      d0: [kf, 128, n_rows] f32   (descriptors0, K-chunked)
      d1: [kf, 128, m_shard] f32  (this core's descriptors1 shard)
    Outputs (per core):
      fwd_val [128, n_chunks*8] f32, fwd_idx [128, n_chunks*8] u32
      bwd_val [128, m_chunks*8] f32, bwd_idx [128, m_chunks*8] u32
    """
    import concourse.bacc as bacc
    import concourse.mybir as mybir
    import concourse.tile as tile

    kf = f_dim // 128
    n_chunks = n_rows // 128     # forward row chunks
    m_tiles = max(1, m_shard // 512)   # 512-wide column tiles per fwd chunk
    mw = min(512, m_shard)
    m_chunks = m_shard // 128    # backward column chunks
    n_tiles = n_rows // 512      # 512-wide row tiles per bwd chunk
    assert m_shard % 128 == 0 and n_rows % 512 == 0 and f_dim % 128 == 0

    nc = bacc.Bacc("TRN2", target_bir_lowering=False, debug=False,
                   num_devices=1)

    d0 = nc.dram_tensor("d0", [kf, 128, n_rows], mybir.dt.float32,
                        kind="ExternalInput")
    d1 = nc.dram_tensor("d1", [kf, 128, m_shard], mybir.dt.float32,
                        kind="ExternalInput")
    fwd_val = nc.dram_tensor("fwd_val", [128, n_chunks * 8], mybir.dt.float32,
                             kind="ExternalOutput")
    fwd_idx = nc.dram_tensor("fwd_idx", [128, n_chunks * 8], mybir.dt.uint32,
                             kind="ExternalOutput")
    bwd_val = nc.dram_tensor("bwd_val", [128, m_chunks * 8], mybir.dt.float32,
                             kind="ExternalOutput")
    bwd_idx = nc.dram_tensor("bwd_idx", [128, m_chunks * 8], mybir.dt.uint32,
                             kind="ExternalOutput")

    with tile.TileContext(nc) as tc:
        with tc.tile_pool(name="persist", bufs=1) as persist, \
             tc.tile_pool(name="schunk", bufs=3) as schunk_pool, \
             tc.tile_pool(name="tchunk", bufs=2) as tchunk_pool, \
             tc.tile_pool(name="outs", bufs=1) as outs_pool, \
             tc.tile_pool(name="psf", bufs=4, space="PSUM") as psf, \
             tc.tile_pool(name="psb", bufs=4, space="PSUM") as psb:

            # resident inputs
            d0_sb = [persist.tile([128, n_rows], mybir.dt.float32,
                                  name=f"d0sb{k}", tag=f"d0sb{k}")
                     for k in range(kf)]
            d1_sb = [persist.tile([128, m_shard], mybir.dt.float32,
                                  name=f"d1sb{k}", tag=f"d1sb{k}")
                     for k in range(kf)]
            for k in range(kf):
                nc.sync.dma_start(d0_sb[k][:], d0[k])
                nc.sync.dma_start(d1_sb[k][:], d1[k])

            fv_sb = outs_pool.tile([128, n_chunks * 8], mybir.dt.float32)
            fi_sb = outs_pool.tile([128, n_chunks * 8], mybir.dt.uint32)
            bv_sb = outs_pool.tile([128, m_chunks * 8], mybir.dt.float32)
            bi_sb = outs_pool.tile([128, m_chunks * 8], mybir.dt.uint32)

            for _rep in range(repeat):
                # ---------------- forward units -------------------------
                def fwd_unit(n):
                    s_chunk = schunk_pool.tile([128, m_shard],
                                               mybir.dt.float32, tag="schunk")
                    for m in range(m_tiles):
                        pf = psf.tile([128, mw], mybir.dt.float32, tag="pf")
                        for k in range(kf):
                            nc.tensor.matmul(
                                pf[:],
                                d0_sb[k][:, n * 128:(n + 1) * 128],
                                d1_sb[k][:, m * mw:(m + 1) * mw],
                                start=(k == 0), stop=(k == kf - 1))
                        nc.scalar.copy(s_chunk[:, m * mw:(m + 1) * mw], pf[:])
                    nc.vector.max(out=fv_sb[:, n * 8:(n + 1) * 8],
                                  in_=s_chunk[:])
                    nc.vector.max_index(out=fi_sb[:, n * 8:(n + 1) * 8],
                                        in_max=fv_sb[:, n * 8:(n + 1) * 8],
                                        in_values=s_chunk[:])

                # ---------------- backward units ------------------------
                def bwd_unit(mm):
                    t_chunk = tchunk_pool.tile([128, n_rows],
                                               mybir.dt.float32, tag="tchunk")
                    for nn in range(n_tiles):
                        pb = psb.tile([128, 512], mybir.dt.float32, tag="pb")
                        for k in range(kf):
                            nc.tensor.matmul(
                                pb[:],
                                d1_sb[k][:, mm * 128:(mm + 1) * 128],
                                d0_sb[k][:, nn * 512:(nn + 1) * 512],
                                start=(k == 0), stop=(k == kf - 1))
                        nc.scalar.copy(t_chunk[:, nn * 512:(nn + 1) * 512],
                                       pb[:])
                    nc.vector.max(out=bv_sb[:, mm * 8:(mm + 1) * 8],
                                  in_=t_chunk[:])
                    nc.vector.max_index(out=bi_sb[:, mm * 8:(mm + 1) * 8],
                                        in_max=bv_sb[:, mm * 8:(mm + 1) * 8],
                                        in_values=t_chunk[:])

                # interleave forward and backward work for engine overlap
                ratio = max(1, n_chunks // max(1, m_chunks))
                mm_next = 0
                for n in range(n_chunks):
                    fwd_unit(n)
                    if (n + 1) % ratio == 0 and mm_next < m_chunks:
                        bwd_unit(mm_next)
                        mm_next += 1
                while mm_next < m_chunks:
                    bwd_unit(mm_next)
                    mm_next += 1

            nc.sync.dma_start(fwd_val[:], fv_sb[:])
            nc.sync.dma_start(fwd_idx[:], fi_sb[:])
            nc.sync.dma_start(bwd_val[:], bv_sb[:])
            nc.sync.dma_start(bwd_idx[:], bi_sb[:])

    nc.compile()
    return nc


_KERNEL_CACHE = {}


def get_kernel(repeat=1):
    key = repeat
    if key not in _KERNEL_CACHE:
        _KERNEL_CACHE[key] = build_kernel(repeat=repeat)
    return _KERNEL_CACHE[key]


# --------------------------------------------------------------------------
# Host side
# --------------------------------------------------------------------------

def _decode_top8(arr, chunks):
    """[128, chunks*8] -> [chunks*128, 8] with row r = chunk*128 + partition."""
    return arr.reshape(128, chunks, 8).transpose(1, 0, 2).reshape(chunks * 128, 8)


def run_device(descriptors0, descriptors1, repeat=1):
    """Run the SPMD kernel on 8 cores. Returns per-core raw outputs."""
    from concourse.bass_utils import run_bass_kernel_spmd

    nc = get_kernel(repeat)
    d0 = np.ascontiguousarray(descriptors0[0]).astype(np.float32, copy=False)
    d1 = np.ascontiguousarray(descriptors1[0]).astype(np.float32, copy=False)
    kf = F_DIM // 128
    d0r = d0.reshape(kf, 128, N_KPTS)
    in_maps = []
    for c in range(N_CORES):
        d1c = np.ascontiguousarray(
            d1[:, c * M_SHARD:(c + 1) * M_SHARD]).reshape(kf, 128, M_SHARD)
        in_maps.append({"d0": d0r, "d1": d1c})
    res = run_bass_kernel_spmd(nc, in_maps, list(range(N_CORES)))
    return res.results


def postprocess(results):
    """Merge per-core device outputs into the reference's 4 output arrays."""
    n = N_KPTS
    n_chunks = n // 128
    m_chunks = M_SHARD // 128

    # ---- forward: merge per-core top-2 into global top-2 ----
    m1 = np.empty((N_CORES, n), np.float32)
    m2 = np.empty((N_CORES, n), np.float32)
    i1 = np.empty((N_CORES, n), np.int64)
    for c in range(N_CORES):
        vals = _decode_top8(results[c]["fwd_val"], n_chunks)
        idxs = _decode_top8(results[c]["fwd_idx"], n_chunks)
        m1[c] = vals[:, 0]
        m2[c] = vals[:, 1]
        i1[c] = idxs[:, 0].astype(np.int64) + c * M_SHARD

    w = np.argmax(m1, axis=0)                      # first max on ties
    rows = np.arange(n)
    s1 = m1[w, rows]
    fwd_nn = i1[w, rows]
    m1_masked = m1.copy()
    m1_masked[w, rows] = -np.inf
    s2 = np.maximum(m1_masked.max(axis=0), m2[w, rows]).astype(np.float32)

    # ---- backward: concatenate per-core full-column results ----
    cm1 = np.empty(M_KPTS, np.float32)
    cm2 = np.empty(M_KPTS, np.float32)
    bck_nn = np.empty(M_KPTS, np.int64)
    for c in range(N_CORES):
        vals = _decode_top8(results[c]["bwd_val"], m_chunks)
        idxs = _decode_top8(results[c]["bwd_idx"], m_chunks)
        sl = slice(c * M_SHARD, (c + 1) * M_SHARD)
        cm1[sl] = vals[:, 0]
        cm2[sl] = vals[:, 1]
        bck_nn[sl] = idxs[:, 0].astype(np.int64)

    # ---- exact reference arithmetic (float32) ----
    def dist(s):
        return SQRT_2 * np.sqrt(np.maximum(ONE - s, CLIP_LO))

    fd1, fd2 = dist(s1), dist(s2)
    fwd_ok = (fd1 / fd2) < np.float32(1.0)
    bd1, bd2 = dist(cm1), dist(cm2)
    bck_ok = (bd1 / bd2) < np.float32(1.0)

    mutual = fwd_ok & bck_ok[fwd_nn] & (bck_nn[fwd_nn] == rows)

    indices0 = np.where(mutual, fwd_nn, -1)[None, :].astype(np.int32)
    mscores0 = (indices0 > 0).astype(np.int32)
    matches1 = np.full((1, M_KPTS), -1, dtype=np.int32)
    mscores1 = np.zeros((1, M_KPTS), dtype=np.float32)
    return indices0, matches1, mscores0, mscores1


def kernel(descriptors0, descriptors1, keypoints0, keypoints1):
    results = run_device(descriptors0, descriptors1)
    return postprocess(results)
